# revision 1
# baseline (speedup 1.0000x reference)
"""3-layer GCN (EnhancedGraphNeuralNetwork) on 8 Trainium2 NeuronCores.

Strategy (dst-node sharded, graph-parallel per the sharding hint):
  - Host: add self loops, compute in-degrees, relabel nodes by descending
    degree, split 128-node blocks round-robin across 8 cores, and pack
    each block's incoming edges into 128-edge chunks (bucketed by 32k-row
    source windows for int16 gather indices). Chunk counts are shared
    across cores so all cores run one SPMD program.
  - Key algebraic fact: aggregation commutes with the layer matmul:
       segsum((dinv*x)[src]) @ W  ==  segsum(((dinv*x) @ W)[src])
    so each layer gathers raw (dinv-scaled) features from a bf16 node
    table via dma_gather, segment-sums chunks into PSUM with one-hot
    selection matmuls (S built on DVE from local dst ids), applies dinv
    on the dst side, transposes, and runs one [128,128]x[128,H] matmul
    per block.
  - BatchNorm: hardware bn_stats per core + 1KB AllReduce.
  - The next layer's node table (dinv * activations, bf16) is rebuilt per
    shard and AllGathered across the 8 cores.
"""

import math
import numpy as np
import ml_dtypes

import concourse.bass as bass
import concourse.bacc as bacc
import concourse.tile as tile
import concourse.mybir as mybir
from concourse.bass_utils import run_bass_kernel_spmd

N_CORES = 8
P = 128
EPS = 1e-5
WIN = 32768          # int16-addressable source window (table rows)
GROUP = 4            # dst blocks per gather group

FP = mybir.dt.float32
BF = mybir.dt.bfloat16
I16 = mybir.dt.int16

PADLOC = 1000.0      # dstloc value for padding entries (kills one-hot row)


# ---------------------------------------------------------------- host prep

def _host_prep(x, edge_index, n_nodes):
    """Relabel, bucket edges by (core, block, window), pack gather plan."""
    N = n_nodes
    NPAD = ((N + (P * N_CORES) - 1) // (P * N_CORES)) * (P * N_CORES)
    J = NPAD // P // N_CORES          # blocks per core
    SH = J * P                        # nodes per core shard
    NW = (NPAD + WIN - 1) // WIN

    src = np.concatenate([edge_index[0], np.arange(N, dtype=np.int64)])
    dst = np.concatenate([edge_index[1], np.arange(N, dtype=np.int64)])

    deg = np.bincount(dst, minlength=N).astype(np.int64)  # >=1 (self loops)
    order = np.argsort(-deg, kind="stable")               # new id -> old id
    newid_of = np.empty(N, dtype=np.int64)
    newid_of[order] = np.arange(N)
    deg_new = np.ones(NPAD, dtype=np.int64)               # pad nodes: deg 1
    deg_new[:N] = deg[order]

    nsrc = newid_of[src]
    ndst = newid_of[dst]

    # table order: node n (new id) -> table row t(n)
    g = np.arange(NPAD) // P
    t_all = (g % N_CORES) * SH + (g // N_CORES) * P + (np.arange(NPAD) % P)

    # edge fields
    e_t = t_all[nsrc]                  # table row of source
    e_w = e_t // WIN                   # source window
    e_rel = (e_t % WIN).astype(np.int32)
    e_g = ndst // P                    # dst global block
    e_c = (e_g % N_CORES).astype(np.int32)
    e_j = (e_g // N_CORES).astype(np.int32)
    e_p = (ndst % P).astype(np.int32)  # dst local id

    # sort edges by (core, block, window) for fast slicing
    key = ((e_c * J + e_j) * NW + e_w).astype(np.int64)
    o = np.argsort(key, kind="stable")
    ks, rels, ps = key[o], e_rel[o], e_p[o]
    bound = np.searchsorted(ks, np.arange(N_CORES * J * NW + 1))

    def seg(c, j, w):
        k = (c * J + j) * NW + w
        return bound[k], bound[k + 1]

    # group blocks
    groups = []
    j0 = 0
    while j0 < J:
        gs = min(GROUP, J - j0)
        groups.append(list(range(j0, j0 + gs)))
        j0 += gs

    # build plan + per-core packed arrays
    plan = []
    idx16 = [[] for _ in range(N_CORES)]   # per core: [128, m/16] int16 parts
    dloc = [[] for _ in range(N_CORES)]    # per core: [128, m/128] f32 parts
    gcol = 0
    for blocks in groups:
        calls = []
        blk_chunks = {j: [] for j in blocks}
        kstart = 0
        for w in range(NW):
            ns = {j: [seg(c, j, w) for c in range(N_CORES)] for j in blocks}
            m_j = {}
            for j in blocks:
                mx = max(b - a for a, b in ns[j])
                m_j[j] = ((mx + P - 1) // P) * P
            nidx = sum(m_j.values())
            if nidx == 0:
                continue
            # split into <=1024-index calls (SWDGE ring limit)
            o16 = sum(c_[2] for c_ in calls)  # int16 cols so far this group
            done = 0
            while done < nidx:
                piece = min(1024, nidx - done)
                calls.append((w, o16 + done // 16, piece // 16,
                              kstart + done // P))
                done += piece
            for c in range(N_CORES):
                vals = np.zeros(nidx, np.int32)
                dls = np.full(nidx, PADLOC, np.float32)
                off = 0
                for j in blocks:
                    a, b = ns[j][c]
                    n = b - a
                    vals[off:off + n] = rels[a:b]
                    dls[off:off + n] = ps[a:b]
                    off += m_j[j]
                wrapped = vals.reshape(nidx // 16, 16).T.astype(np.int16)
                idx16[c].append(np.tile(wrapped, (8, 1)))
                dloc[c].append(dls.reshape(nidx // P, P).T)
            boff = 0
            for j in blocks:
                nch = m_j[j] // P
                for i in range(nch):
                    blk_chunks[j].append(kstart + boff // P + i)
                boff += m_j[j]
            kstart += nidx // P
            gcol += nidx // P
        plan.append(dict(blocks=blocks, calls=calls, slots=kstart,
                         chunks=blk_chunks))

    idx16 = np.stack([np.concatenate(idx16[c], axis=1)
                      for c in range(N_CORES)])
    dloc = np.stack([np.concatenate(dloc[c], axis=1) for c in range(N_CORES)])

    # per-core shard data in table order
    xs = np.zeros((N_CORES, SH, x.shape[1]), dtype=np.float32)
    degt = np.ones((N_CORES, P, J), dtype=np.float32)
    for c in range(N_CORES):
        gbs = np.arange(J) * N_CORES + c
        nids = (gbs[:, None] * P + np.arange(P)[None, :]).reshape(-1)
        real = nids < N
        xr = np.zeros((SH, x.shape[1]), dtype=np.float32)
        xr[real] = x[order[nids[real]]]
        xs[c] = xr
        degt[c] = deg_new[nids].reshape(J, P).T.astype(np.float32)

    meta = dict(N=N, NPAD=NPAD, J=J, SH=SH, NW=NW, plan=plan,
                o16_total=idx16.shape[2], slots_total=dloc.shape[2],
                order=order)
    return meta, idx16, dloc.astype(ml_dtypes.bfloat16), xs, degt


# ---------------------------------------------------------------- device

def _build(meta, hid, n_cls, stage=99, ldepth=99):
    """Build the SPMD bass program for all 8 cores."""
    J, SH = meta["J"], meta["SH"]
    N, NPAD = meta["N"], meta["NPAD"]
    plan = meta["plan"]
    F = hid
    O16, SLOTS = meta["o16_total"], meta["slots_total"]
    SLOTS_MAX = max(pl["slots"] for pl in plan)
    O16_MAX = max(sum(c[2] for c in pl["calls"]) for pl in plan)

    nc = bacc.Bacc("TRN2", target_bir_lowering=False, debug=False,
                   num_devices=N_CORES)

    xsh = nc.dram_tensor("xsh", [SH, F], FP, kind="ExternalInput")
    degt_d = nc.dram_tensor("degt", [P, J], FP, kind="ExternalInput")
    idx_d = nc.dram_tensor("idx16", [P, O16], I16, kind="ExternalInput")
    dloc_d = nc.dram_tensor("dloc", [P, SLOTS], BF, kind="ExternalInput")
    ident_d = nc.dram_tensor("ident", [P, P], FP, kind="ExternalInput")
    iota_d = nc.dram_tensor("iotar", [P, P], BF, kind="ExternalInput")
    W1_d = nc.dram_tensor("W1", [F, F], FP, kind="ExternalInput")
    W2_d = nc.dram_tensor("W2", [F, F], FP, kind="ExternalInput")
    W3_d = nc.dram_tensor("W3", [F, n_cls], FP, kind="ExternalInput")
    b1_d = nc.dram_tensor("b1", [F], FP, kind="ExternalInput")
    b2_d = nc.dram_tensor("b2", [F], FP, kind="ExternalInput")
    b3_d = nc.dram_tensor("b3", [n_cls], FP, kind="ExternalInput")
    g1_d = nc.dram_tensor("g1", [F], FP, kind="ExternalInput")
    be1_d = nc.dram_tensor("be1", [F], FP, kind="ExternalInput")
    g2_d = nc.dram_tensor("g2", [F], FP, kind="ExternalInput")
    be2_d = nc.dram_tensor("be2", [F], FP, kind="ExternalInput")
    out_d = nc.dram_tensor("out", [P, J, n_cls], FP, kind="ExternalOutput")

    with tile.TileContext(nc) as tc:
        with (
            tc.tile_pool(name="persist", bufs=1) as pp,
            tc.tile_pool(name="blk", bufs=3) as bp,
            tc.tile_pool(name="spool", bufs=4) as sp,
            tc.tile_pool(name="gath", bufs=2) as gp,
            tc.tile_pool(name="psum", bufs=2, space="PSUM") as psp,
            tc.tile_pool(name="dram", bufs=1, space="DRAM") as dp,
        ):
            # ---------- constants
            ident = pp.tile([P, P], FP, tag="ident")
            nc.sync.dma_start(out=ident[:], in_=ident_d[:])
            ident_bf = pp.tile([P, P], BF, tag="identbf")
            nc.vector.tensor_copy(out=ident_bf[:], in_=ident[:])
            iotar = pp.tile([P, P], BF, tag="iotar")
            nc.sync.dma_start(out=iotar[:], in_=iota_d[:])

            w1 = pp.tile([F, F], BF, tag="w1")
            w2 = pp.tile([F, F], BF, tag="w2")
            w3 = pp.tile([F, n_cls], BF, tag="w3")
            nc.gpsimd.dma_start(out=w1[:], in_=W1_d[:])
            nc.gpsimd.dma_start(out=w2[:], in_=W2_d[:])
            nc.gpsimd.dma_start(out=w3[:], in_=W3_d[:])

            def col(dram1d, n=F):
                t = pp.tile([n, 1], FP, tag=f"col_{dram1d.name}")
                nc.sync.dma_start(out=t[:], in_=dram1d[:, None])
                return t

            b1c, b2c = col(b1_d), col(b2_d)
            g1c, be1c, g2c, be2c = col(g1_d), col(be1_d), col(g2_d), col(be2_d)
            b3bc = pp.tile([P, n_cls], FP, tag="b3bc")
            nc.gpsimd.dma_start(
                out=b3bc[:],
                in_=bass.AP(tensor=b3_d, offset=0, ap=[[0, P], [1, n_cls]]))
            epsc = pp.tile([P, 1], FP, tag="eps")
            nc.vector.memset(epsc[:], EPS)

            degt = pp.tile([P, J], FP, tag="degt")
            nc.sync.dma_start(out=degt[:], in_=degt_d[:])
            dinv = pp.tile([P, J], FP, tag="dinv")
            nc.scalar.activation(out=dinv[:], in_=degt[:],
                                 func=mybir.ActivationFunctionType.Sqrt)
            nc.vector.reciprocal(out=dinv[:], in_=dinv[:])

            # ---------- big persistent buffers (xT/A/Z3 share one slot)
            xT = pp.tile([F, SH], FP, tag="bigbuf")        # residual (f32)
            Z = pp.tile([F, SH], FP, tag="z")              # pre-BN activations
            A = pp.tile([F, SH], BF, tag="bigbuf")         # post-act (bf16)
            Z3 = pp.tile([P, J, n_cls], FP, tag="bigbuf")

            # ---------- DRAM internals
            agin = dp.tile([SH, F], BF, tag="agin")
            tables = [dp.tile([NPAD, F], BF, tag=f"table{i}",
                              name=f"table{i}", addr_space="Shared")
                      for i in range(3)]
            st_in = dp.tile([P, 2], FP, tag="stin")
            st_outs = [dp.tile([P, 2], FP, tag=f"stout{i}",
                               name=f"stout{i}", addr_space="Shared")
                       for i in range(2)]

            # ---------- layer-1 table prep: agin = dinv * x (bf16), xT
            for j in range(J):
                xblk = bp.tile([P, F], FP, tag="xblk")
                nc.sync.dma_start(out=xblk[:], in_=xsh[j * P:(j + 1) * P, :])
                xsc = bp.tile([P, F], BF, tag="xsc")
                nc.vector.tensor_scalar_mul(out=xsc[:], in0=xblk[:],
                                            scalar1=dinv[:, j:j + 1])
                nc.sync.dma_start(out=agin[j * P:(j + 1) * P, :], in_=xsc[:])
                xtp = psp.tile([P, P], FP, tag="ps_t")
                nc.tensor.transpose(out=xtp[:], in_=xblk[:], identity=ident[:])
                nc.vector.tensor_copy(out=xT[:, j * P:(j + 1) * P], in_=xtp[:])

            def allgather_table(li):
                nc.gpsimd.collective_compute(
                    "AllGather", mybir.AluOpType.bypass,
                    replica_groups=[list(range(N_CORES))],
                    ins=[agin[:]], outs=[tables[li][:]])

            allgather_table(0)

            # ---------- one GCN layer
            def layer(w_sb, out_h, bias_col, z_dst, li):
                table = tables[li]
                o16_base = 0
                col_base = 0
                for pl in plan:
                    slots = pl["slots"]
                    o16_len = sum(c[2] for c in pl["calls"])
                    idx_sb = gp.tile([P, O16_MAX], I16, tag="idxsb")
                    nc.sync.dma_start(
                        out=idx_sb[:, :o16_len],
                        in_=idx_d[:, o16_base:o16_base + o16_len])
                    dl_sb = gp.tile([P, SLOTS_MAX], BF, tag="dlsb")
                    nc.sync.dma_start(
                        out=dl_sb[:, :slots],
                        in_=dloc_d[:, col_base:col_base + slots])
                    strip = gp.tile([P, SLOTS_MAX, F], BF, tag="strip")
                    for (w, o16, n16, kstart) in pl["calls"]:
                        nidx = n16 * 16
                        lo = w * WIN
                        hi = min(NPAD, lo + WIN)
                        nc.gpsimd.dma_gather(
                            out_ap=strip[:, kstart:kstart + nidx // P, :],
                            in_ap=table[lo:hi, :],
                            idxs_ap=idx_sb[:, o16:o16 + n16],
                            num_idxs=nidx, num_idxs_reg=nidx, elem_size=F)
                    for j in pl["blocks"]:
                        if ldepth < 1:
                            continue
                        chunks = pl["chunks"][j]
                        pagg = psp.tile([P, P], FP, tag="ps_agg")
                        nch = len(chunks)
                        for i, t_in in enumerate(chunks):
                            S = sp.tile([P, P], BF, tag="S")
                            nc.vector.tensor_tensor(
                                out=S[:],
                                in0=dl_sb[:, t_in:t_in + 1].to_broadcast([P, P]),
                                in1=iotar[:], op=mybir.AluOpType.is_equal)
                            nc.tensor.matmul(pagg[:], lhsT=S[:],
                                             rhs=strip[:, t_in, :],
                                             start=(i == 0), stop=(i == nch - 1))
                        aggs = bp.tile([P, F], BF, tag="aggs")
                        nc.vector.tensor_scalar_mul(out=aggs[:], in0=pagg[:],
                                                    scalar1=dinv[:, j:j + 1])
                        if ldepth < 2:
                            continue
                        pt = psp.tile([P, P], BF, tag="ps_tb")
                        nc.tensor.transpose(out=pt[:], in_=aggs[:],
                                            identity=ident_bf[:])
                        aggT = bp.tile([P, F], BF, tag="aggT")
                        nc.vector.tensor_copy(out=aggT[:], in_=pt[:])
                        if ldepth < 3:
                            continue
                        pz = psp.tile([P, out_h], FP, tag="ps_z")
                        if out_h == n_cls:
                            nc.tensor.matmul(pz[:], lhsT=aggT[:], rhs=w_sb[:],
                                             start=True, stop=True)
                            nc.vector.tensor_add(out=z_dst[:, j, :], in0=pz[:],
                                                 in1=b3bc[:])
                        else:
                            nc.tensor.matmul(pz[:], lhsT=w_sb[:], rhs=aggT[:],
                                             start=True, stop=True)
                            nc.vector.tensor_scalar(
                                out=z_dst[:, j * P:(j + 1) * P], in0=pz[:],
                                scalar1=bias_col[:], scalar2=None,
                                op0=mybir.AluOpType.add)
                    o16_base += o16_len
                    col_base += slots

            # ---------- BN (global) + act; writes A (bf16)
            def bn_relu(g_col, be_col, residual, li):
                st_out = st_outs[li]
                sub = math.gcd(512, SH)
                nsub = SH // sub
                stats = bp.tile([P, nsub, 6], FP, tag="bnst")
                zv = Z[:].rearrange("p (s q) -> p s q", s=nsub)
                for s in range(nsub):
                    nc.vector.bn_stats(out=stats[:, s, :], in_=zv[:, s, :])
                mv = bp.tile([P, 2], FP, tag="bnmv")
                nc.vector.bn_aggr(out=mv[:], in_=stats[:])
                sums = bp.tile([P, 2], FP, tag="sums")
                musq = bp.tile([P, 1], FP, tag="musq")
                nc.vector.tensor_mul(out=musq[:], in0=mv[:, 0:1], in1=mv[:, 0:1])
                nc.scalar.mul(out=sums[:, 0:1], in_=mv[:, 0:1], mul=float(SH))
                nc.vector.tensor_add(out=sums[:, 1:2], in0=mv[:, 1:2],
                                     in1=musq[:])
                nc.scalar.mul(out=sums[:, 1:2], in_=sums[:, 1:2], mul=float(SH))
                nc.sync.dma_start(out=st_in[:], in_=sums[:])
                nc.gpsimd.collective_compute(
                    "AllReduce", mybir.AluOpType.add,
                    replica_groups=[list(range(N_CORES))],
                    ins=[st_in[:]], outs=[st_out[:]])
                gl = bp.tile([P, 2], FP, tag="gl")
                nc.sync.dma_start(out=gl[:], in_=st_out[:])
                mu = bp.tile([P, 1], FP, tag="mu")
                var = bp.tile([P, 1], FP, tag="var")
                nc.scalar.mul(out=mu[:], in_=gl[:, 0:1], mul=1.0 / N)
                nc.scalar.mul(out=var[:], in_=gl[:, 1:2], mul=1.0 / N)
                nc.vector.tensor_mul(out=musq[:], in0=mu[:], in1=mu[:])
                nc.vector.tensor_sub(out=var[:], in0=var[:], in1=musq[:])
                rstd = bp.tile([P, 1], FP, tag="rstd")
                nc.scalar.activation(out=rstd[:], in_=var[:],
                                     func=mybir.ActivationFunctionType.Sqrt,
                                     bias=epsc[:], scale=1.0)
                nc.vector.reciprocal(out=rstd[:], in_=rstd[:])
                sc = bp.tile([P, 1], FP, tag="sc")
                sh = bp.tile([P, 1], FP, tag="sh")
                nc.vector.tensor_mul(out=sc[:], in0=g_col[:], in1=rstd[:])
                nc.vector.tensor_mul(out=sh[:], in0=mu[:], in1=sc[:])
                nc.vector.tensor_sub(out=sh[:], in0=be_col[:], in1=sh[:])
                nc.vector.tensor_scalar(out=Z[:], in0=Z[:], scalar1=sc[:],
                                        scalar2=sh[:],
                                        op0=mybir.AluOpType.mult,
                                        op1=mybir.AluOpType.add)
                if residual:
                    nc.vector.tensor_add(out=Z[:], in0=Z[:], in1=xT[:])
                nc.scalar.activation(out=A[:], in_=Z[:],
                                     func=mybir.ActivationFunctionType.Relu)

            # ---------- next-layer table: agin = dinv * A (per block)
            def table_prep():
                for j in range(J):
                    pt = psp.tile([P, P], BF, tag="ps_tb")
                    nc.tensor.transpose(out=pt[:], in_=A[:, j * P:(j + 1) * P],
                                        identity=ident_bf[:])
                    ts = bp.tile([P, F], BF, tag="tps")
                    nc.vector.tensor_scalar_mul(out=ts[:], in0=pt[:],
                                                scalar1=dinv[:, j:j + 1])
                    nc.sync.dma_start(out=agin[j * P:(j + 1) * P, :],
                                      in_=ts[:])

            # ================= layers
            def _early_out():
                nc.vector.memset(Z3[:], 0.0)
                nc.vector.tensor_add(out=Z3[:, 0, :1], in0=Z[:, 0:1],
                                     in1=Z[:, 1:2])
                nc.sync.dma_start(out=out_d[:], in_=Z3[:])

            if stage >= 2:
                nc.vector.memset(Z[:], 0.0)
                layer(w1, F, b1c, Z, 0)
            else:
                nc.vector.memset(Z[:], 0.0)
            if stage >= 3:
                bn_relu(g1c, be1c, residual=True, li=0)
            if stage >= 4:
                table_prep()
                allgather_table(1)
            if stage >= 5:
                layer(w2, F, b2c, Z, 1)
                bn_relu(g2c, be2c, residual=False, li=1)
                table_prep()
                allgather_table(2)
            if stage >= 6:
                layer(w3, n_cls, None, Z3, 2)

            if stage < 6:
                _early_out()
                do_softmax = False
            else:
                do_softmax = True
            # ---------- log_softmax over classes (free dim)
            zv = Z3[:]                                    # [P, J, C]
            if do_softmax:
                mx = bp.tile([P, J, 1], FP, tag="mx")
                nc.vector.reduce_max(out=mx[:], in_=zv,
                                     axis=mybir.AxisListType.X)
                nc.vector.tensor_sub(out=zv, in0=zv,
                                     in1=mx[:].to_broadcast([P, J, n_cls]))
                ex = pp.tile([P, J, n_cls], FP, tag="z")   # Z is dead here
                nc.scalar.activation(out=ex[:], in_=zv,
                                     func=mybir.ActivationFunctionType.Exp)
                sm = bp.tile([P, J, 1], FP, tag="sm")
                nc.vector.reduce_sum(out=sm[:], in_=ex[:],
                                     axis=mybir.AxisListType.X)
                ls = bp.tile([P, J, 1], FP, tag="ls")
                nc.scalar.activation(out=ls[:], in_=sm[:],
                                     func=mybir.ActivationFunctionType.Ln)
                nc.vector.tensor_sub(out=zv, in0=zv,
                                     in1=ls[:].to_broadcast([P, J, n_cls]))
                nc.sync.dma_start(out=out_d[:], in_=Z3[:])

    nc.compile()
    return nc


def _make_in_maps(meta, idx16, dloc, xs, degt, inputs):
    iota_rows = np.tile(np.arange(P, dtype=np.float32)[None, :], (P, 1))
    shared = dict(
        ident=np.eye(P, dtype=np.float32),
        iotar=iota_rows.astype(ml_dtypes.bfloat16),
        **{k: np.asarray(inputs[k], np.float32)
           for k in ("W1", "W2", "W3", "b1", "b2", "b3",
                     "g1", "be1", "g2", "be2")})
    return [dict(xsh=xs[c], degt=degt[c], idx16=idx16[c], dloc=dloc[c],
                 **shared) for c in range(N_CORES)]


def _unshard(meta, results, n_cls):
    J, SH = meta["J"], meta["SH"]
    out = np.empty((meta["NPAD"], n_cls), np.float32)
    for c in range(N_CORES):
        o = results[c]["out"]                             # [P, J, C]
        nids = ((np.arange(J) * N_CORES + c)[:, None] * P
                + np.arange(P)[None, :])
        out[nids.reshape(-1)] = o.transpose(1, 0, 2).reshape(SH, n_cls)
    full = np.empty((meta["N"], n_cls), np.float32)
    full[meta["order"]] = out[:meta["N"]]
    return full


# ---------------------------------------------------------------- entry

def kernel(x, edge_index, W1, b1, g1, be1, W2, b2, g2, be2, W3, b3):
    x = np.asarray(x, dtype=np.float32)
    edge_index = np.asarray(edge_index)
    N, F = x.shape
    C = np.asarray(W3).shape[1]

    meta, idx16, dloc, xs, degt = _host_prep(x, edge_index, N)
    nc = _build(meta, F, C)
    in_maps = _make_in_maps(meta, idx16, dloc, xs, degt, dict(
        W1=W1, W2=W2, W3=W3, b1=b1, b2=b2, b3=b3,
        g1=g1, be1=be1, g2=g2, be2=be2))
    res = run_bass_kernel_spmd(nc, in_maps, core_ids=list(range(N_CORES)))
    return _unshard(meta, res.results, C)

